# revision 1
# baseline (speedup 1.0000x reference)
"""Ewald real-space potential kernel for Trainium2 (8 NeuronCores, SPMD).

Computes pot = 0.5 * sum_{i != j} q_i * erf(d_ij / sqrt(2)) / d_ij * C  over
all pairs of N=4096 atoms, C = 90.0474 / (2*pi).

Sharding: each core owns a 512-column i-slice of the 4096x4096 pair matrix.
On-chip layout per work tile is [128 j-partitions, 512 i-free]; j-blocks are
processed in super-groups of 6 (one [128, 3072] PSUM tensor, PE fills one
3-block half while ACT reads the other via bank-level dependency tracking):
  1. PE     : d2 = |r_i - r_j|^2 via a K=16 fp16 hi/lo-split matmul
              (fp16 products are exact in fp32 PSUM; |err| < 8e-3). The
              core's own j-window carries +0.01 on |r_j|^2 so the masked
              diagonal stays positive under that error; real pairs are
              unaffected (closest pair d2 = 0.0144).
  2. ACT    : dist = Sqrt(d2)            (PSUM -> SBUF, one op per super)
  3. DVE    : rinv = reciprocal_approx_fast(dist)   (~51 ULP)
  4. ACT    : erf_t = Erf(dist / sqrt(2))
  5. DVE    : zero erf_t on the self-interaction diagonal (mask multiply)
  6. DVE/GP : w = erf_t * rinv   (cast fp16, kept resident in SBUF)
  7. PE     : s[1,512] += q_j^T @ w  -- all 32 fp16 matmuls emitted after the
              elementwise loop so the PE FIFO never blocks d2 production.
  8. DVE/ACT: pot = C/2 * sum_i q_i * s_i
The j-axis is rotated per core so each core's own diagonal window lands in
j-blocks 28..31, letting one shared mask tensor serve all cores (SPMD).
Host sums the 8 per-core partial potentials.
"""

import numpy as np

P = 128
N = 4096
NCORES = 8
COLS = N // NCORES          # 512 i-columns per core
NB = N // P                 # 32 j-blocks
K = 16                      # split-matmul contraction rows
SIGMA = 1.0
NORM_CONST = 90.0474 / (2.0 * np.pi)
A = 1.0 / (SIGMA * np.sqrt(2.0))
DIAG_EPS = 0.01             # |r_j|^2 bias on the core's own j-window only
# super-groups: j-blocks per elementwise op; last group covers the remainder
SUPERS = [6, 6, 6, 6, 6, 2]
# which super-groups run the final w-multiply on gpsimd (rest on vector)
GPSIMD_SUPERS = frozenset({4, 5})

_CACHE = {}


def _split2(v32):
    h = v32.astype(np.float16)
    l = (v32 - h.astype(np.float32)).astype(np.float16)
    return h, l


def _build_core_inputs(q, r):
    """Per-core input arrays (keyed by dram tensor name)."""
    q = q.astype(np.float32)
    r = r.astype(np.float32)
    r2_64 = (r.astype(np.float64) ** 2).sum(1)

    # shared diagonal mask: Z[p, u] = 0 iff u == p + 384  (sliced per j-block)
    z = np.ones((P, 896), np.float32)
    z[np.arange(P), np.arange(P) + 384] = 0.0

    in_maps = []
    for c in range(NCORES):
        perm = (np.arange(N) + COLS * (c + 1)) % N  # rotated j order
        win = slice(COLS * c, COLS * (c + 1))       # this core's i-window

        rows_j, rows_i = [], []
        for d in range(3):
            cj = r[perm, d]
            ui = (-2.0 * r[win, d]).astype(np.float32)
            jh, jl = _split2(cj)
            ih, il = _split2(ui)
            rows_j += [jh, jh, jl, jl]
            rows_i += [ih, il, ih, il]
        r2j = r2_64[perm].copy()
        r2j[N - COLS:] += DIAG_EPS      # rotated slots 3584.. = own window
        r2j = r2j.astype(np.float32)
        r2i = r2_64[win].astype(np.float32)
        jh, jl = _split2(r2j)
        ih, il = _split2(r2i)
        ones_j = np.ones(N, np.float16)
        ones_i = np.ones(COLS, np.float16)
        rows_j += [jh, jl, ones_j, ones_j]
        rows_i += [ones_i, ones_i, ih, il]

        in_maps.append({
            "aj": np.stack(rows_j).astype(np.float16),          # [K, N]
            "bi": np.stack(rows_i).astype(np.float16),          # [K, COLS]
            "qj": q[perm].reshape(NB, P).T.astype(np.float16),  # [P, NB]
            "qi": (q[win] * (0.5 * NORM_CONST)).reshape(1, COLS)
                  .astype(np.float32),                          # [1, COLS]
            "zmask": z,                                         # [P, 896]
        })
    return in_maps


def _build_program():
    import concourse.mybir as mybir
    import concourse.tile as tile
    from concourse import bacc

    dt = mybir.dt
    nc = bacc.Bacc("TRN2", target_bir_lowering=False, debug=False,
                   num_devices=NCORES)

    aj = nc.dram_tensor("aj", [K, N], dt.float16, kind="ExternalInput")
    bi = nc.dram_tensor("bi", [K, COLS], dt.float16, kind="ExternalInput")
    qj = nc.dram_tensor("qj", [P, NB], dt.float16, kind="ExternalInput")
    qi = nc.dram_tensor("qi", [1, COLS], dt.float32, kind="ExternalInput")
    zmask = nc.dram_tensor("zmask", [P, 896], dt.float32, kind="ExternalInput")
    pot = nc.dram_tensor("pot", [1, 1], dt.float32, kind="ExternalOutput")

    erf_fn = mybir.ActivationFunctionType.Erf
    sqrt_fn = mybir.ActivationFunctionType.Sqrt
    SW = 6 * COLS   # super-group width (elements in free dim)

    with tile.TileContext(nc) as tc:
        with (
            tc.tile_pool(name="const", bufs=1) as cpool,
            tc.tile_pool(name="work", bufs=3) as wpool,
            tc.tile_pool(name="distk", bufs=len(SUPERS)) as dpool,
            tc.tile_pool(name="rinvk", bufs=len(SUPERS)) as rpool,
            tc.tile_pool(name="d2pool", bufs=2, space="PSUM") as ppool,
            tc.tile_pool(name="spool", bufs=1, space="PSUM") as spool,
            tc.tile_pool(name="warmp", bufs=1, space="PSUM") as warmpool,
        ):
            AJ = cpool.tile([K, N], dt.float16)
            nc.sync.dma_start(AJ[:, :N // 4], aj[:, :N // 4])
            nc.sync.dma_start(AJ[:, N // 4:], aj[:, N // 4:])
            BI = cpool.tile([K, COLS], dt.float16)
            nc.sync.dma_start(BI[:], bi[:])
            QJ = cpool.tile([P, NB], dt.float16)
            nc.gpsimd.dma_start(QJ[:], qj[:])
            QI = cpool.tile([1, COLS], dt.float32)
            nc.gpsimd.dma_start(QI[:], qi[:])
            ZM = cpool.tile([P, 896], dt.float32)
            nc.gpsimd.dma_start(ZM[:], zmask[:])

            s_ps = spool.tile([1, COLS], dt.float32)

            # ---- phase A: d2 matmuls + Sqrt (one ACT table) + recip ----
            # PSUM: two [128, 1536] d2 buffers (3 banks each) so PE fills one
            # while ACT drains the other; dist/rinv are wide [128, 3072]
            # tiles written in halves and consumed whole in phase B.
            dist_tiles, rinv_tiles = [], []
            jb0 = 0
            for g, gsz in enumerate(SUPERS):
                fd = gsz * COLS
                dist = dpool.tile([P, SW], dt.float32, tag="dist")
                rinv = rpool.tile([P, SW], dt.float32, tag="rinv")
                nh = max(1, gsz // 3)
                hsz = gsz // nh
                for h in range(nh):
                    c0 = h * hsz * COLS
                    d2 = ppool.tile([P, 3 * COLS], dt.float32, tag="d2")
                    for k in range(hsz):
                        jb = jb0 + h * hsz + k
                        nc.tensor.matmul(
                            d2[:, k * COLS:(k + 1) * COLS],
                            AJ[:, jb * P:(jb + 1) * P],
                            BI[:, :],
                            start=True, stop=True,
                        )
                    nc.scalar.activation(dist[:, c0:c0 + hsz * COLS],
                                         d2[:, :hsz * COLS], sqrt_fn)
                nc.vector.reciprocal_approx_fast(rinv[:, :fd], dist[:, :fd])
                # zero the self-pair diagonal via rinv on the otherwise-idle
                # gpsimd engine (j-blocks 28..31 after rotation)
                for k in range(gsz):
                    jb = jb0 + k
                    if jb >= NB - 4:
                        b = jb - (NB - 4)
                        off = (3 - b) * P
                        nc.gpsimd.tensor_mul(
                            rinv[:, k * COLS:(k + 1) * COLS],
                            rinv[:, k * COLS:(k + 1) * COLS],
                            ZM[:, off:off + COLS],
                        )
                dist_tiles.append(dist)
                rinv_tiles.append(rinv)
                jb0 += gsz

            # keep the PE's HAM clock warm across its idle window at the
            # start of phase B (cold-rate matmuls are ~2x slower)
            wrm = cpool.tile([P, COLS], dt.float16)
            nc.gpsimd.tensor_copy(wrm[:, :], dist_tiles[4][:, :COLS])
            warm_ps = warmpool.tile([1, COLS], dt.float32)
            for _ in range(14):
                nc.tensor.matmul(warm_ps[:, :], QJ[:, 0:1], wrm[:, :],
                                 start=True, stop=True)

            # ---- phase B: Erf (single table switch), w, reduction ----
            # Erf writes to PSUM (recycling the d2 slots): ACT's writes and
            # DVE's erf reads leave SBUF, removing port contention with the
            # concurrent w-multiplies.
            jb0 = 0
            for g, gsz in enumerate(SUPERS):
                fd = gsz * COLS
                dist = dist_tiles[g]
                rinv = rinv_tiles[g]
                wa = wpool.tile([P, SW], dt.float16, tag="wa")
                nh = max(1, gsz // 3)
                hsz = gsz // nh
                for h in range(nh):
                    c0 = h * hsz * COLS
                    hw = hsz * COLS
                    erf_ps = ppool.tile([P, 3 * COLS], dt.float32, tag="d2")
                    nc.scalar.activation(erf_ps[:, :hw], dist[:, c0:c0 + hw],
                                         erf_fn, scale=float(A))
                    nc.vector.tensor_mul(wa[:, c0:c0 + hw], erf_ps[:, :hw],
                                         rinv[:, c0:c0 + hw])
                    for k in range(hsz):
                        jb = jb0 + h * hsz + k
                        nc.tensor.matmul(
                            s_ps[:, :],
                            QJ[:, jb:jb + 1],
                            wa[:, (h * hsz + k) * COLS:(h * hsz + k + 1) * COLS],
                            start=(jb == 0), stop=(jb == NB - 1),
                        )
                jb0 += gsz

            sq = cpool.tile([1, COLS], dt.float32)
            pot_sb = cpool.tile([1, 1], dt.float32)
            nc.vector.scalar_tensor_tensor(
                out=sq[:, :], in0=s_ps[:, :], scalar=1.0, in1=QI[:, :],
                op0=mybir.AluOpType.mult, op1=mybir.AluOpType.mult,
                accum_out=pot_sb[:, :],
            )
            nc.sync.dma_start(pot[:, :], pot_sb[:, :])

    nc.compile()
    return nc


def _get_program():
    if "nc" not in _CACHE:
        _CACHE["nc"] = _build_program()
    return _CACHE["nc"]


def _run(q, r, trace=False, **trace_kwargs):
    from concourse.bass_utils import run_bass_kernel_spmd

    nc = _get_program()
    in_maps = _build_core_inputs(np.asarray(q), np.asarray(r))
    res = run_bass_kernel_spmd(nc, in_maps, core_ids=list(range(NCORES)),
                               trace=trace, **trace_kwargs)
    total = np.float64(0.0)
    for m in res.results:
        total += np.float64(m["pot"].reshape(-1)[0])
    return np.array([total], dtype=np.float32), res


def kernel(q, r, cell=None, batch=None):
    out, _ = _run(q, r, trace=False)
    return out



# revision 5
# speedup vs baseline: 1.4150x; 1.4150x over previous
"""Ewald real-space potential kernel for Trainium2 (8 NeuronCores, SPMD).

Computes pot = 0.5 * sum_{i != j} q_i * erf(d_ij / sqrt(2)) / d_ij * C over
all pairs of N=4096 atoms, C = 90.0474 / (2*pi).

v2 design — symmetry + single-pass elementwise chain:

1. Symmetric cover (1.78x less work than the full matrix): each core owns a
   512-column i-window and 20 j-blocks of 128 rows:
     blocks  0..11 : groups c+1, c+2, c+3 (full width, weight 1)
     blocks 12..15 : own group c (diagonal, weight 1/2, masked diagonal)
     blocks 16..19 : antipodal group c+-4, quadrant-split half width
   Every unordered atom pair is covered exactly once (diag pairs twice at 1/2).

2. Elementwise chain is ONE ACT pass + ONE custom-DVE pass:
     w(d2) = C*erf(a*d)/d  with a = 1/sqrt(2).
   erf(x)/x is even in x => analytic in s = d2, so the near field is a plain
   polynomial in s. Far field (s > ~10) has erf == 1 => w = C/sqrt(s).
   Device computes  w' = min(P4(t), rinv')  where t = beta*s is produced
   directly by the d2 matmul (coords pre-scaled by sqrt(beta)),
   P4(t) = t^4 + B3 t^3 + B2 t^2 + B1 t + 1 is the monic-normalized quartic
   minimax fit, and rinv' = rsqrt(lam*t) = (C/c0)/sqrt(s) comes from a single
   Abs_reciprocal_sqrt activation (one table set, no erf/sqrt tables).
   The quartic is constrained to stay above C/sqrt(s) past the crossover, so
   min() performs the branch select. Global scale c0 is folded into QI.

3. PE: K=13 fp16 hi/lo-split matmul for t = beta*d2 (lo*lo terms dropped),
   plus per-block [128,1] reduce matmuls  s += qj^T @ w'  (weights folded
   into QJ). Final stt computes pot = sum_i s_i * (c0*q_i) per core; host
   sums the 8 partials in float64.
"""

import numpy as np

P = 128
N = 4096
NCORES = 8
COLS = N // NCORES          # 512 i-columns per core
K = 13                      # split-matmul contraction rows
NBLK = 20                   # j-blocks per core (16 full + 4 half width)
SIGMA = 1.0
NORM_CONST = 90.0474 / (2.0 * np.pi)
DIAG_EPS = 0.01

# Constrained minimax quartic fit of C*erf(a*sqrt(s))/sqrt(s) on s in [0, 10]
# (LP: |Q-w|<=t on [0,10]; Q>=w on [10,16]; Q>=C/sqrt(s) on [16, 5200]).
QF = (1.14266144e+01, -1.85816289e+00, 2.41013009e-01,
      -1.81744163e-02, 5.76130940e-04)
BETA = float((QF[4] / QF[0]) ** 0.25)          # t = BETA * d2
B1 = float(QF[1] / QF[0] / BETA)
B2 = float(QF[2] / QF[0] / BETA ** 2)
B3 = float(QF[3] / QF[0] / BETA ** 3)
LAM = float((QF[0] / NORM_CONST) ** 2 / BETA)  # rsqrt(LAM*t) = (C/c0)/sqrt(s)

# super-groups of j-blocks; each is 1536 elems wide in the free dim
SUPERS = ((0, 1, 2), (3, 4, 5), (6, 7, 8), (9, 10, 11), (12, 13, 14),
          (15, 16, 17, 18, 19))
# moving-operand column window per block (quadrant split for blocks 16..19)
BLK_CSLICE = [(0, 512)] * 16 + [(0, 256), (0, 256), (256, 512), (256, 512)]

_CACHE = {}


def _split2(v32):
    h = v32.astype(np.float16)
    l = (v32 - h.astype(np.float32)).astype(np.float16)
    return h, l


def _core_jblocks(c):
    """[(j_atom_start, weight, diag_idx or None), ...] for the 20 blocks."""
    blocks = []
    for k in (1, 2, 3):
        g = (c + k) % NCORES
        blocks += [(g * COLS + b * P, 1.0, None) for b in range(4)]
    blocks += [(c * COLS + b * P, 0.5, b) for b in range(4)]
    g = (c + 4) % NCORES
    order = (0, 1, 2, 3) if c < 4 else (2, 3, 0, 1)
    blocks += [(g * COLS + b * P, 1.0, None) for b in order]
    return blocks


def _build_core_inputs(q, r):
    q = q.astype(np.float32)
    r = r.astype(np.float32)
    r2_64 = (r.astype(np.float64) ** 2).sum(1)
    sb = np.sqrt(BETA)

    # shared diagonal mask: Z[p, u] = 0 iff u == p + 384; diag block b uses
    # slice [384-128b : 896-128b] so the zero lands at column 128b + p.
    z = np.ones((P, 896), np.float16)
    z[np.arange(P), np.arange(P) + 384] = 0.0

    in_maps = []
    for c in range(NCORES):
        win = slice(COLS * c, COLS * (c + 1))
        blocks = _core_jblocks(c)
        jidx = np.concatenate([np.arange(js, js + P) for js, _, _ in blocks])

        rows_j, rows_i = [], []
        for d in range(3):
            cj = (sb * r[jidx, d]).astype(np.float32)
            ui = (-2.0 * sb * r[win, d]).astype(np.float32)
            jh, jl = _split2(cj)
            ih, il = _split2(ui)
            rows_j += [jh, jh, jl]
            rows_i += [ih, il, ih]
        r2j = BETA * r2_64[jidx]
        for bi, (js, _, diag_b) in enumerate(blocks):
            if diag_b is not None:
                r2j[bi * P:(bi + 1) * P] += BETA * DIAG_EPS
        jh, jl = _split2(r2j.astype(np.float32))
        ih, il = _split2((BETA * r2_64[win]).astype(np.float32))
        ones_j = np.ones(NBLK * P, np.float16)
        ones_i = np.ones(COLS, np.float16)
        rows_j += [jh, jl, ones_j, ones_j]
        rows_i += [ones_i, ones_i, ih, il]

        qj = np.stack([q[js:js + P] * w for js, w, _ in blocks], 1)

        in_maps.append({
            "aj": np.stack(rows_j).astype(np.float16),          # [K, 2560]
            "bi": np.stack(rows_i).astype(np.float16),          # [K, COLS]
            "qj": qj.astype(np.float16),                        # [P, NBLK]
            "qi": (q[win] * QF[0]).reshape(1, COLS)
                  .astype(np.float32),                          # [1, COLS]
            "zmask": z,                                         # [P, 896]
        })
    return in_maps


def _register_qmin():
    """Register the quartic+min custom DVE op (row 17, v3) once."""
    import concourse.dve_ops as dvo
    from concourse.dve_spec import (
        C0, C1, C2, One, Spec, Src0, Src1, lower, minn,
    )
    from concourse.dve_uop import DveOpSpec

    name = "TENSOR_QUARTIC_MIN_EW"
    if name in dvo._SUB_OPCODE_FOR_NAME:
        return dvo.CUSTOM_DVE_OP_QMIN_EW

    t = Src0
    body = minn(((((t + C2) * t + C1) * t + C0) * t) + One, Src1)

    def ref(in0, in1, s0, s1, imm2):
        tt = in0.astype(np.float32)
        h = ((((tt + imm2) * tt + s1) * tt + s0) * tt) + np.float32(1.0)
        return np.minimum(h, in1.astype(np.float32)).astype(np.float32)

    spec = Spec(body=body, reference=ref)
    row = max(dvo._SUB_OPCODE_FOR_NAME.values()) + 1
    dvo._SUB_OPCODE_FOR_NAME[name] = row
    shas = {}
    for ver in ("v3",):
        uops = lower(spec, ver=ver)
        shas[ver] = DveOpSpec(name=name, opcode=row, uops=uops,
                              rd1_en=True).sha(ver)
    op = dvo.DveOp(name=name, spec=spec, subdim=False, uops_sha=shas)
    dvo.OPS.append(op)
    dvo.CUSTOM_DVE_SPECS[name] = spec
    dvo.CUSTOM_DVE_OP_QMIN_EW = op
    return op


def _build_program():
    import concourse.mybir as mybir
    import concourse.tile as tile
    from concourse import bacc

    qmin_op = _register_qmin()

    dt = mybir.dt
    nc = bacc.Bacc("TRN2", target_bir_lowering=False, debug=False,
                   num_devices=NCORES)

    aj = nc.dram_tensor("aj", [K, NBLK * P], dt.float16, kind="ExternalInput")
    bi = nc.dram_tensor("bi", [K, COLS], dt.float16, kind="ExternalInput")
    qj = nc.dram_tensor("qj", [P, NBLK], dt.float16, kind="ExternalInput")
    qi = nc.dram_tensor("qi", [1, COLS], dt.float32, kind="ExternalInput")
    zmask = nc.dram_tensor("zmask", [P, 896], dt.float16, kind="ExternalInput")
    pot = nc.dram_tensor("pot", [1, 1], dt.float32, kind="ExternalOutput")

    rsq_fn = mybir.ActivationFunctionType.Abs_reciprocal_sqrt
    SW = 3 * COLS  # super width

    with tile.TileContext(nc) as tc:
        with (
            tc.tile_pool(name="const", bufs=1) as cpool,
            tc.tile_pool(name="wts", bufs=3) as wpool,
            tc.tile_pool(name="rinvs", bufs=2) as rpool,
            tc.tile_pool(name="d2pool", bufs=2, space="PSUM") as ppool,
            tc.tile_pool(name="spool", bufs=1, space="PSUM") as spool,
        ):
            AJ = cpool.tile([K, NBLK * P], dt.float16)
            nc.sync.dma_start(AJ[:, :NBLK * P // 2], aj[:, :NBLK * P // 2])
            nc.sync.dma_start(AJ[:, NBLK * P // 2:], aj[:, NBLK * P // 2:])
            BI = cpool.tile([K, COLS], dt.float16)
            nc.sync.dma_start(BI[:], bi[:])
            QJ = cpool.tile([P, NBLK], dt.float16)
            nc.gpsimd.dma_start(QJ[:], qj[:])
            QI = cpool.tile([1, COLS], dt.float32)
            nc.gpsimd.dma_start(QI[:], qi[:])
            ZM = cpool.tile([P, 896], dt.float16)
            nc.gpsimd.dma_start(ZM[:], zmask[:])

            s_ps = spool.tile([1, COLS], dt.float32)
            s2 = spool.tile([1, COLS], dt.float32)

            w_tiles = []

            def emit_reduce(g):
                wt, offs = w_tiles[g]
                for b, off in zip(SUPERS[g], offs):
                    lo, hi = BLK_CSLICE[b]
                    wslice = wt[:, off:off + (hi - lo)]
                    if b < 16:
                        nc.tensor.matmul(s_ps[:, :], QJ[:, b:b + 1], wslice,
                                         start=(b == 0), stop=(b == 15))
                    else:
                        nc.tensor.matmul(s2[:, lo:hi], QJ[:, b:b + 1], wslice,
                                         start=(b in (16, 18)),
                                         stop=(b in (17, 19)),
                                         skip_group_check=True)

            for g, blks in enumerate(SUPERS):
                d2 = ppool.tile([P, SW], dt.float32, tag="d2")
                off = 0
                offs = []
                for b in blks:
                    lo, hi = BLK_CSLICE[b]
                    w = hi - lo
                    nc.tensor.matmul(
                        d2[:, off:off + w],
                        AJ[:, b * P:(b + 1) * P],
                        BI[:, lo:hi],
                        start=True, stop=True,
                    )
                    offs.append(off)
                    off += w
                rinv = rpool.tile([P, SW], dt.float32, tag="rinv")
                nc.scalar.activation(rinv[:, :], d2[:, :], rsq_fn,
                                     scale=float(LAM))
                wt = wpool.tile([P, SW], dt.float16, tag="wt")
                nc.vector._custom_dve(qmin_op, out=wt[:, :], in0=d2[:, :],
                                      in1=rinv[:, :], s0=float(B1),
                                      s1=float(B2), imm2=float(B3))
                # zero the self-pair diagonal on the otherwise-idle gpsimd
                for b, o in zip(blks, offs):
                    if 12 <= b <= 15:
                        bd = b - 12
                        zoff = 384 - 128 * bd
                        nc.gpsimd.tensor_mul(
                            wt[:, o:o + COLS], wt[:, o:o + COLS],
                            ZM[:, zoff:zoff + COLS],
                        )
                w_tiles.append((wt, offs))
                if g >= 1:
                    emit_reduce(g - 1)
            emit_reduce(len(SUPERS) - 1)

            sq = cpool.tile([1, COLS], dt.float32)
            sq2 = cpool.tile([1, COLS], dt.float32)
            pot_a = cpool.tile([1, 1], dt.float32)
            pot_b = cpool.tile([1, 1], dt.float32)
            pot_sb = cpool.tile([1, 1], dt.float32)
            nc.vector.scalar_tensor_tensor(
                out=sq[:, :], in0=s_ps[:, :], scalar=1.0, in1=QI[:, :],
                op0=mybir.AluOpType.mult, op1=mybir.AluOpType.mult,
                accum_out=pot_a[:, :],
            )
            nc.vector.scalar_tensor_tensor(
                out=sq2[:, :], in0=s2[:, :], scalar=1.0, in1=QI[:, :],
                op0=mybir.AluOpType.mult, op1=mybir.AluOpType.mult,
                accum_out=pot_b[:, :],
            )
            nc.vector.tensor_add(pot_sb[:, :], pot_a[:, :], pot_b[:, :])
            nc.sync.dma_start(pot[:, :], pot_sb[:, :])

    nc.compile()
    return nc


def _get_program():
    if "nc" not in _CACHE:
        _CACHE["nc"] = _build_program()
    return _CACHE["nc"]


def _run(q, r, trace=False, **trace_kwargs):
    from concourse.bass_utils import run_bass_kernel_spmd

    nc = _get_program()
    in_maps = _build_core_inputs(np.asarray(q), np.asarray(r))
    res = run_bass_kernel_spmd(nc, in_maps, core_ids=list(range(NCORES)),
                               trace=trace, **trace_kwargs)
    total = np.float64(0.0)
    for m in res.results:
        total += np.float64(m["pot"].reshape(-1)[0])
    return np.array([total], dtype=np.float32), res


def kernel(q, r, cell=None, batch=None):
    out, _ = _run(q, r, trace=False)
    return out


# revision 8
# speedup vs baseline: 1.5098x; 1.0670x over previous
"""Ewald real-space potential kernel for Trainium2 (8 NeuronCores, SPMD).

Computes pot = 0.5 * sum_{i != j} q_i * erf(d_ij / sqrt(2)) / d_ij * C over
all pairs of N=4096 atoms, C = 90.0474 / (2*pi).

v3 design — symmetry + single-pass elementwise chain, no diagonal mask:

1. Symmetric cover (1.78x less work than the full matrix): each core owns a
   512-column i-window and 20 j-blocks of 128 rows:
     blocks  0..11 : groups c+1, c+2, c+3 (full width, weight 1)
     blocks 12..15 : own group c (diagonal, weight 1/2)
     blocks 16..19 : antipodal group c+-4, quadrant-split half width
   Every unordered atom pair is covered exactly once (diag pairs twice at 1/2).

2. Elementwise chain is ONE ACT pass + ONE custom-DVE pass:
     w(d2) = C*erf(a*d)/d  with a = 1/sqrt(2).
   erf(x)/x is even in x => analytic in s = d2, so the near field is a plain
   polynomial in s. Far field (s > ~10) has erf == 1 => w = C/sqrt(s).
   Device computes  w' = min(P4(t), rinv')  where t = beta*s comes straight
   from the d2 matmul (coords pre-scaled by sqrt(beta)),
   P4(t) = t^4 + B3 t^3 + B2 t^2 + B1 t + 1 is the monic-normalized quartic
   minimax fit and rinv' = rsqrt(lam*t) = (C/c0)/sqrt(s) is a single
   Abs_reciprocal_sqrt activation (one table set, no erf/sqrt tables).
   The fit is constrained to stay above C/sqrt(s) past the crossover so
   min() performs the branch select. Global scale c0 folds into QI.

3. Self-pairs are NOT masked: the diagonal is biased to d2 = DIAG_EPS (the bias rides the r2j
   row, so it must stay small vs real pair distances), and the known
   contribution sum_i 0.5*q_i^2*c0*P4(beta*eps) is subtracted on the host.

4. PE: K=13 fp16 hi/lo-split matmul for t (lo*lo terms dropped) plus
   [128,1]-stationary reduce matmuls  s += qj^T @ w'  (pair weights folded
   into QJ). A warm-up matmul burst at program start ramps the PE clock to
   its 2.4 GHz p-state before the real work; small fillers hold it there.
   Final stt computes pot = sum_i s_i * (c0*q_i); host sums the 8 partials
   and subtracts the diagonal term in float64.
"""

import numpy as np

P = 128
N = 4096
NCORES = 8
COLS = N // NCORES          # 512 i-columns per core
K = 13                      # split-matmul contraction rows
NBLK = 20                   # j-blocks per core (16 full + 4 half width)
SIGMA = 1.0
NORM_CONST = 90.0474 / (2.0 * np.pi)
DIAG_EPS = 0.01

# Constrained minimax quartic fit of C*erf(a*sqrt(s))/sqrt(s) on s in [0, 10]
# (LP: |Q-w|<=t on [0,10]; Q>=w on [10,16]; Q>=C/sqrt(s) on [16, 5200]).
QF = (1.14266144e+01, -1.85816289e+00, 2.41013009e-01,
      -1.81744163e-02, 5.76130940e-04)
BETA = float((QF[4] / QF[0]) ** 0.25)          # t = BETA * d2
B1 = float(QF[1] / QF[0] / BETA)
B2 = float(QF[2] / QF[0] / BETA ** 2)
B3 = float(QF[3] / QF[0] / BETA ** 3)
LAM = float((QF[0] / NORM_CONST) ** 2 / BETA)  # rsqrt(LAM*t) = (C/c0)/sqrt(s)


def _p4(t):
    return ((((t + B3) * t + B2) * t + B1) * t) + 1.0


# super-groups of j-blocks; each is 1536 elems wide in the free dim
SUPERS = ((0, 1, 2), (3, 4, 5), (6, 7, 8), (9, 10, 11), (12, 13, 14),
          (15, 16, 17, 18, 19))
# moving-operand column window per block (quadrant split for blocks 16..19)
BLK_CSLICE = [(0, 512)] * 16 + [(0, 256), (0, 256), (256, 512), (256, 512)]
N_WARM = 8                  # startup PE warm-up matmuls ([1,512] each)

_CACHE = {}


def _split2(v32):
    h = v32.astype(np.float16)
    l = (v32 - h.astype(np.float32)).astype(np.float16)
    return h, l


def _core_jblocks(c):
    """[(j_atom_start, weight, is_diag), ...] for the 20 blocks."""
    blocks = []
    for k in (1, 2, 3):
        g = (c + k) % NCORES
        blocks += [(g * COLS + b * P, 1.0, False) for b in range(4)]
    blocks += [(c * COLS + b * P, 0.5, True) for b in range(4)]
    g = (c + 4) % NCORES
    order = (0, 1, 2, 3) if c < 4 else (2, 3, 0, 1)
    blocks += [(g * COLS + b * P, 1.0, False) for b in order]
    return blocks


def _build_core_inputs(q, r):
    q = q.astype(np.float32)
    r = r.astype(np.float32)
    r2_64 = (r.astype(np.float64) ** 2).sum(1)
    sb = np.sqrt(BETA)

    in_maps = []
    for c in range(NCORES):
        win = slice(COLS * c, COLS * (c + 1))
        blocks = _core_jblocks(c)
        jidx = np.concatenate([np.arange(js, js + P) for js, _, _ in blocks])

        rows_j, rows_i = [], []
        for d in range(3):
            cj = (sb * r[jidx, d]).astype(np.float32)
            ui = (-2.0 * sb * r[win, d]).astype(np.float32)
            jh, jl = _split2(cj)
            ih, il = _split2(ui)
            rows_j += [jh, jh, jl]
            rows_i += [ih, il, ih]
        r2j = BETA * r2_64[jidx]
        for bi, (js, _, is_diag) in enumerate(blocks):
            if is_diag:
                r2j[bi * P:(bi + 1) * P] += BETA * DIAG_EPS
        jh, jl = _split2(r2j.astype(np.float32))
        ih, il = _split2((BETA * r2_64[win]).astype(np.float32))
        ones_j = np.ones(NBLK * P, np.float16)
        ones_i = np.ones(COLS, np.float16)
        rows_j += [jh, jl, ones_j, ones_j]
        rows_i += [ones_i, ones_i, ih, il]

        qj = np.stack([q[js:js + P] * w for js, w, _ in blocks], 1)

        in_maps.append({
            "aj": np.stack(rows_j).astype(np.float16),          # [K, 2560]
            "bi": np.stack(rows_i).astype(np.float16),          # [K, COLS]
            "qj": qj.astype(np.float16),                        # [P, NBLK]
            "qi": (q[win] * QF[0]).reshape(1, COLS)
                  .astype(np.float32),                          # [1, COLS]
        })
    return in_maps


def _diag_correction(q):
    """Exactly-known self-pair term the device sums: 0.5*q_i^2*c0*P4(b*eps)."""
    q64 = np.asarray(q, np.float64)
    qj16 = (0.5 * np.asarray(q, np.float32)).astype(np.float16)
    w_diag = _p4(BETA * DIAG_EPS)
    return float(np.sum(qj16.astype(np.float64) * q64) * QF[0] * w_diag)


def _register_qmin():
    """Register the quartic+min custom DVE op (row 17, v3) once."""
    import concourse.dve_ops as dvo
    from concourse.dve_spec import (
        C0, C1, C2, One, Spec, Src0, Src1, lower, minn,
    )
    from concourse.dve_uop import DveOpSpec

    name = "TENSOR_QUARTIC_MIN_EW"
    if name in dvo._SUB_OPCODE_FOR_NAME:
        return dvo.CUSTOM_DVE_OP_QMIN_EW

    t = Src0
    body = minn(((((t + C2) * t + C1) * t + C0) * t) + One, Src1)

    def ref(in0, in1, s0, s1, imm2):
        tt = in0.astype(np.float32)
        h = ((((tt + imm2) * tt + s1) * tt + s0) * tt) + np.float32(1.0)
        return np.minimum(h, in1.astype(np.float32)).astype(np.float32)

    spec = Spec(body=body, reference=ref)
    row = max(dvo._SUB_OPCODE_FOR_NAME.values()) + 1
    dvo._SUB_OPCODE_FOR_NAME[name] = row
    shas = {}
    for ver in ("v3",):
        uops = lower(spec, ver=ver)
        shas[ver] = DveOpSpec(name=name, opcode=row, uops=uops,
                              rd1_en=True).sha(ver)
    op = dvo.DveOp(name=name, spec=spec, subdim=False, uops_sha=shas)
    dvo.OPS.append(op)
    dvo.CUSTOM_DVE_SPECS[name] = spec
    dvo.CUSTOM_DVE_OP_QMIN_EW = op
    return op


def _build_program():
    import concourse.mybir as mybir
    import concourse.tile as tile
    from concourse import bacc

    qmin_op = _register_qmin()

    dt = mybir.dt
    nc = bacc.Bacc("TRN2", target_bir_lowering=False, debug=False,
                   num_devices=NCORES)

    aj = nc.dram_tensor("aj", [K, NBLK * P], dt.float16, kind="ExternalInput")
    bi = nc.dram_tensor("bi", [K, COLS], dt.float16, kind="ExternalInput")
    qj = nc.dram_tensor("qj", [P, NBLK], dt.float16, kind="ExternalInput")
    qi = nc.dram_tensor("qi", [1, COLS], dt.float32, kind="ExternalInput")
    pot = nc.dram_tensor("pot", [1, 1], dt.float32, kind="ExternalOutput")

    rsq_fn = mybir.ActivationFunctionType.Abs_reciprocal_sqrt
    SW = 3 * COLS  # super width

    with tile.TileContext(nc) as tc:
        with (
            tc.tile_pool(name="const", bufs=1) as cpool,
            tc.tile_pool(name="wts", bufs=3) as wpool,
            tc.tile_pool(name="rinvs", bufs=2) as rpool,
            tc.tile_pool(name="d2pool", bufs=2, space="PSUM") as ppool,
            tc.tile_pool(name="spool", bufs=1, space="PSUM") as spool,
        ):
            # PE warm-up: ramp the tensor engine to its fast p-state while
            # the input DMAs are in flight (no data dependencies).
            W0 = cpool.tile([P, COLS], dt.float16)
            nc.gpsimd.memset(W0[:, :], 0.0)
            warm_ps = spool.tile([1, COLS], dt.float32)
            for _ in range(N_WARM):
                nc.tensor.matmul(warm_ps[:, :], W0[:, 0:1], W0[:, :],
                                 start=True, stop=True)

            BI = cpool.tile([K, COLS], dt.float16)
            nc.sync.dma_start(BI[:], bi[:])
            AJ = cpool.tile([K, NBLK * P], dt.float16)
            nc.sync.dma_start(AJ[:], aj[:])
            QJ = cpool.tile([P, NBLK], dt.float16)
            nc.gpsimd.dma_start(QJ[:], qj[:])
            QI = cpool.tile([1, COLS], dt.float32)
            nc.gpsimd.dma_start(QI[:], qi[:])

            s_ps = spool.tile([1, COLS], dt.float32)

            w_tiles = []

            def emit_reduce(g):
                wt, offs = w_tiles[g]
                for b, off in zip(SUPERS[g], offs):
                    lo, hi = BLK_CSLICE[b]
                    nc.tensor.matmul(s_ps[:, lo:hi], QJ[:, b:b + 1],
                                     wt[:, off:off + (hi - lo)],
                                     start=(b == 0), stop=(b in (17, 19)),
                                     skip_group_check=(b >= 15))

            for g, blks in enumerate(SUPERS):
                d2 = ppool.tile([P, SW], dt.float32, tag="d2")
                off = 0
                offs = []
                for b in blks:
                    lo, hi = BLK_CSLICE[b]
                    w = hi - lo
                    nc.tensor.matmul(
                        d2[:, off:off + w],
                        AJ[:, b * P:(b + 1) * P],
                        BI[:, lo:hi],
                        start=True, stop=True,
                    )
                    offs.append(off)
                    off += w
                rinv = rpool.tile([P, SW], dt.float32, tag="rinv")
                nc.scalar.activation(rinv[:, :], d2[:, :], rsq_fn,
                                     scale=float(LAM))
                wt = wpool.tile([P, SW], dt.float16, tag="wt")
                nc.vector._custom_dve(qmin_op, out=wt[:, :], in0=d2[:, :],
                                      in1=rinv[:, :], s0=float(B1),
                                      s1=float(B2), imm2=float(B3))
                w_tiles.append((wt, offs))
                if g >= 1:
                    # keep the PE p-state hot across the producer wait
                    nc.tensor.matmul(warm_ps[:, :384], W0[:, 0:1],
                                     W0[:, :384], start=True, stop=True)
                    emit_reduce(g - 1)
            emit_reduce(len(SUPERS) - 1)

            sq = cpool.tile([1, COLS], dt.float32)
            pot_sb = cpool.tile([1, 1], dt.float32)
            nc.vector.scalar_tensor_tensor(
                out=sq[:, :], in0=s_ps[:, :], scalar=1.0, in1=QI[:, :],
                op0=mybir.AluOpType.mult, op1=mybir.AluOpType.mult,
                accum_out=pot_sb[:, :],
            )
            nc.sync.dma_start(pot[:, :], pot_sb[:, :])

    nc.compile()
    return nc


def _get_program():
    if "nc" not in _CACHE:
        _CACHE["nc"] = _build_program()
    return _CACHE["nc"]


def _run(q, r, trace=False, **trace_kwargs):
    from concourse.bass_utils import run_bass_kernel_spmd

    nc = _get_program()
    in_maps = _build_core_inputs(np.asarray(q), np.asarray(r))
    res = run_bass_kernel_spmd(nc, in_maps, core_ids=list(range(NCORES)),
                               trace=trace, **trace_kwargs)
    total = np.float64(0.0)
    for m in res.results:
        total += np.float64(m["pot"].reshape(-1)[0])
    total -= _diag_correction(q)
    return np.array([total], dtype=np.float32), res


def kernel(q, r, cell=None, batch=None):
    out, _ = _run(q, r, trace=False)
    return out


# revision 9
# speedup vs baseline: 1.6943x; 1.1222x over previous
"""Ewald real-space potential kernel for Trainium2 (8 NeuronCores, SPMD).

Computes pot = 0.5 * sum_{i != j} q_i * erf(d_ij / sqrt(2)) / d_ij * C over
all pairs of N=4096 atoms, C = 90.0474 / (2*pi).

v3 design — symmetry + single-pass elementwise chain, no diagonal mask:

1. Symmetric cover (1.78x less work than the full matrix): each core owns a
   512-column i-window and 20 j-blocks of 128 rows:
     blocks  0..11 : groups c+1, c+2, c+3 (full width, weight 1)
     blocks 12..15 : own group c (diagonal, weight 1/2)
     blocks 16..19 : antipodal group c+-4, quadrant-split half width
   Every unordered atom pair is covered exactly once (diag pairs twice at 1/2).

2. Elementwise chain is ONE ACT pass + ONE custom-DVE pass:
     w(d2) = C*erf(a*d)/d  with a = 1/sqrt(2).
   erf(x)/x is even in x => analytic in s = d2, so the near field is a plain
   polynomial in s. Far field (s > ~10) has erf == 1 => w = C/sqrt(s).
   Device computes  w' = min(P4(t), rinv')  where t = beta*s comes straight
   from the d2 matmul (coords pre-scaled by sqrt(beta)),
   P4(t) = t^4 + B3 t^3 + B2 t^2 + B1 t + 1 is the monic-normalized quartic
   minimax fit and rinv' = rsqrt(lam*t) = (C/c0)/sqrt(s) is a single
   Abs_reciprocal_sqrt activation (one table set, no erf/sqrt tables).
   The fit is constrained to stay above C/sqrt(s) past the crossover so
   min() performs the branch select. Global scale c0 folds into QI.

3. Self-pairs are NOT masked: the diagonal is biased to d2 = DIAG_EPS (the bias rides the r2j
   row, so it must stay small vs real pair distances), and the known
   contribution sum_i 0.5*q_i^2*c0*P4(beta*eps) is subtracted on the host.

4. PE: K=13 fp16 hi/lo-split matmul for t (lo*lo terms dropped) plus
   [128,1]-stationary reduce matmuls  s += qj^T @ w'  (pair weights folded
   into QJ). A warm-up matmul burst at program start ramps the PE clock to
   its 2.4 GHz p-state before the real work; small fillers hold it there.
   Final stt computes pot = sum_i s_i * (c0*q_i); host sums the 8 partials
   and subtracts the diagonal term in float64.
"""

import numpy as np

P = 128
N = 4096
NCORES = 8
COLS = N // NCORES          # 512 i-columns per core
K = 13                      # split-matmul contraction rows
NBLK = 20                   # j-blocks per core (16 full + 4 half width)
SIGMA = 1.0
NORM_CONST = 90.0474 / (2.0 * np.pi)
DIAG_EPS = 0.01

# Constrained minimax quartic fit of C*erf(a*sqrt(s))/sqrt(s) on s in [0, 10]
# (LP: |Q-w|<=t on [0,10]; Q>=w on [10,16]; Q>=C/sqrt(s) on [16, 5200]).
QF = (1.14266144e+01, -1.85816289e+00, 2.41013009e-01,
      -1.81744163e-02, 5.76130940e-04)
BETA = float((QF[4] / QF[0]) ** 0.25)          # t = BETA * d2
B1 = float(QF[1] / QF[0] / BETA)
B2 = float(QF[2] / QF[0] / BETA ** 2)
B3 = float(QF[3] / QF[0] / BETA ** 3)
LAM = float((QF[0] / NORM_CONST) ** 2 / BETA)  # rsqrt(LAM*t) = (C/c0)/sqrt(s)


def _p4(t):
    return ((((t + B3) * t + B2) * t + B1) * t) + 1.0


# super-groups of j-blocks; each is 1024 elems wide in the free dim
SUPERS = tuple((2 * i, 2 * i + 1) for i in range(8)) + ((16, 17, 18, 19),)
# moving-operand column window per block (quadrant split for blocks 16..19)
BLK_CSLICE = [(0, 512)] * 16 + [(0, 256), (0, 256), (256, 512), (256, 512)]
N_WARM = 3                  # startup PE warm-up matmuls ([1,512] each)

_CACHE = {}


def _split2(v32):
    h = v32.astype(np.float16)
    l = (v32 - h.astype(np.float32)).astype(np.float16)
    return h, l


def _core_jblocks(c):
    """[(j_atom_start, weight, is_diag), ...] for the 20 blocks."""
    blocks = []
    for k in (1, 2, 3):
        g = (c + k) % NCORES
        blocks += [(g * COLS + b * P, 1.0, False) for b in range(4)]
    blocks += [(c * COLS + b * P, 0.5, True) for b in range(4)]
    g = (c + 4) % NCORES
    order = (0, 1, 2, 3) if c < 4 else (2, 3, 0, 1)
    blocks += [(g * COLS + b * P, 1.0, False) for b in order]
    return blocks


def _build_core_inputs(q, r):
    q = q.astype(np.float32)
    r = r.astype(np.float32)
    r2_64 = (r.astype(np.float64) ** 2).sum(1)
    sb = np.sqrt(BETA)

    in_maps = []
    for c in range(NCORES):
        win = slice(COLS * c, COLS * (c + 1))
        blocks = _core_jblocks(c)
        jidx = np.concatenate([np.arange(js, js + P) for js, _, _ in blocks])

        rows_j, rows_i = [], []
        for d in range(3):
            cj = (sb * r[jidx, d]).astype(np.float32)
            ui = (-2.0 * sb * r[win, d]).astype(np.float32)
            jh, jl = _split2(cj)
            ih, il = _split2(ui)
            rows_j += [jh, jh, jl]
            rows_i += [ih, il, ih]
        r2j = BETA * r2_64[jidx]
        for bi, (js, _, is_diag) in enumerate(blocks):
            if is_diag:
                r2j[bi * P:(bi + 1) * P] += BETA * DIAG_EPS
        jh, jl = _split2(r2j.astype(np.float32))
        ih, il = _split2((BETA * r2_64[win]).astype(np.float32))
        ones_j = np.ones(NBLK * P, np.float16)
        ones_i = np.ones(COLS, np.float16)
        rows_j += [jh, jl, ones_j, ones_j]
        rows_i += [ones_i, ones_i, ih, il]

        qj = np.stack([q[js:js + P] * w for js, w, _ in blocks], 1)

        in_maps.append({
            "aj": np.stack(rows_j).astype(np.float16),          # [K, 2560]
            "bi": np.stack(rows_i).astype(np.float16),          # [K, COLS]
            "qj": qj.astype(np.float16),                        # [P, NBLK]
            "qi": (q[win] * QF[0]).reshape(1, COLS)
                  .astype(np.float32),                          # [1, COLS]
        })
    return in_maps


def _diag_correction(q):
    """Exactly-known self-pair term the device sums: 0.5*q_i^2*c0*P4(b*eps)."""
    q64 = np.asarray(q, np.float64)
    qj16 = (0.5 * np.asarray(q, np.float32)).astype(np.float16)
    w_diag = _p4(BETA * DIAG_EPS)
    return float(np.sum(qj16.astype(np.float64) * q64) * QF[0] * w_diag)


def _register_qmin():
    """Register the quartic+min custom DVE op (row 17, v3) once."""
    import concourse.dve_ops as dvo
    from concourse.dve_spec import (
        C0, C1, C2, One, Spec, Src0, Src1, lower, minn,
    )
    from concourse.dve_uop import DveOpSpec

    name = "TENSOR_QUARTIC_MIN_EW"
    if name in dvo._SUB_OPCODE_FOR_NAME:
        return dvo.CUSTOM_DVE_OP_QMIN_EW

    t = Src0
    body = minn(((((t + C2) * t + C1) * t + C0) * t) + One, Src1)

    def ref(in0, in1, s0, s1, imm2):
        tt = in0.astype(np.float32)
        h = ((((tt + imm2) * tt + s1) * tt + s0) * tt) + np.float32(1.0)
        return np.minimum(h, in1.astype(np.float32)).astype(np.float32)

    spec = Spec(body=body, reference=ref)
    row = max(dvo._SUB_OPCODE_FOR_NAME.values()) + 1
    dvo._SUB_OPCODE_FOR_NAME[name] = row
    shas = {}
    for ver in ("v3",):
        uops = lower(spec, ver=ver)
        shas[ver] = DveOpSpec(name=name, opcode=row, uops=uops,
                              rd1_en=True).sha(ver)
    op = dvo.DveOp(name=name, spec=spec, subdim=False, uops_sha=shas)
    dvo.OPS.append(op)
    dvo.CUSTOM_DVE_SPECS[name] = spec
    dvo.CUSTOM_DVE_OP_QMIN_EW = op
    return op


def _build_program():
    import concourse.mybir as mybir
    import concourse.tile as tile
    from concourse import bacc

    qmin_op = _register_qmin()

    dt = mybir.dt
    nc = bacc.Bacc("TRN2", target_bir_lowering=False, debug=False,
                   num_devices=NCORES)

    aj = nc.dram_tensor("aj", [K, NBLK * P], dt.float16, kind="ExternalInput")
    bi = nc.dram_tensor("bi", [K, COLS], dt.float16, kind="ExternalInput")
    qj = nc.dram_tensor("qj", [P, NBLK], dt.float16, kind="ExternalInput")
    qi = nc.dram_tensor("qi", [1, COLS], dt.float32, kind="ExternalInput")
    pot = nc.dram_tensor("pot", [1, 1], dt.float32, kind="ExternalOutput")

    rsq_fn = mybir.ActivationFunctionType.Abs_reciprocal_sqrt
    SW = 2 * COLS  # super width

    with tile.TileContext(nc) as tc:
        with (
            tc.tile_pool(name="const", bufs=1) as cpool,
            tc.tile_pool(name="wts", bufs=4) as wpool,
            tc.tile_pool(name="rinvs", bufs=3) as rpool,
            tc.tile_pool(name="d2pool", bufs=3, space="PSUM") as ppool,
            tc.tile_pool(name="spool", bufs=1, space="PSUM") as spool,
        ):
            # PE warm-up: ramp the tensor engine to its fast p-state while
            # the input DMAs are in flight (no data dependencies).
            W0 = cpool.tile([P, COLS], dt.float16)
            nc.gpsimd.memset(W0[:, :], 0.0)
            warm_ps = spool.tile([1, COLS], dt.float32)
            for _ in range(N_WARM):
                nc.tensor.matmul(warm_ps[:, :], W0[:, 0:1], W0[:, :],
                                 start=True, stop=True)

            AJ = cpool.tile([K, NBLK * P], dt.float16)
            nc.sync.dma_start(AJ[:], aj[:])
            BI = cpool.tile([K, COLS], dt.float16)
            nc.gpsimd.dma_start(BI[:], bi[:])
            QJ = cpool.tile([P, NBLK], dt.float16)
            nc.gpsimd.dma_start(QJ[:], qj[:])
            QI = cpool.tile([1, COLS], dt.float32)
            nc.gpsimd.dma_start(QI[:], qi[:])

            s_ps = spool.tile([1, COLS], dt.float32)

            w_tiles = []

            def emit_reduce(g):
                wt, offs = w_tiles[g]
                for b, off in zip(SUPERS[g], offs):
                    lo, hi = BLK_CSLICE[b]
                    nc.tensor.matmul(s_ps[:, lo:hi], QJ[:, b:b + 1],
                                     wt[:, off:off + (hi - lo)],
                                     start=(b == 0), stop=(b in (17, 19)),
                                     skip_group_check=(b >= 15))

            for g, blks in enumerate(SUPERS):
                d2 = ppool.tile([P, SW], dt.float32, tag="d2")
                off = 0
                offs = []
                for b in blks:
                    lo, hi = BLK_CSLICE[b]
                    w = hi - lo
                    nc.tensor.matmul(
                        d2[:, off:off + w],
                        AJ[:, b * P:(b + 1) * P],
                        BI[:, lo:hi],
                        start=True, stop=True,
                    )
                    offs.append(off)
                    off += w
                rinv = rpool.tile([P, SW], dt.float32, tag="rinv")
                nc.scalar.activation(rinv[:, :], d2[:, :], rsq_fn,
                                     scale=float(LAM))
                wt = wpool.tile([P, SW], dt.float16, tag="wt")
                nc.vector._custom_dve(qmin_op, out=wt[:, :], in0=d2[:, :],
                                      in1=rinv[:, :], s0=float(B1),
                                      s1=float(B2), imm2=float(B3))
                w_tiles.append((wt, offs))
                if g >= 2:
                    emit_reduce(g - 2)
            emit_reduce(len(SUPERS) - 2)
            emit_reduce(len(SUPERS) - 1)

            sq = cpool.tile([1, COLS], dt.float32)
            pot_sb = cpool.tile([1, 1], dt.float32)
            nc.vector.scalar_tensor_tensor(
                out=sq[:, :], in0=s_ps[:, :], scalar=1.0, in1=QI[:, :],
                op0=mybir.AluOpType.mult, op1=mybir.AluOpType.mult,
                accum_out=pot_sb[:, :],
            )
            nc.sync.dma_start(pot[:, :], pot_sb[:, :])

    nc.compile()
    return nc


def _get_program():
    if "nc" not in _CACHE:
        _CACHE["nc"] = _build_program()
    return _CACHE["nc"]


def _run(q, r, trace=False, **trace_kwargs):
    from concourse.bass_utils import run_bass_kernel_spmd

    nc = _get_program()
    in_maps = _build_core_inputs(np.asarray(q), np.asarray(r))
    res = run_bass_kernel_spmd(nc, in_maps, core_ids=list(range(NCORES)),
                               trace=trace, **trace_kwargs)
    total = np.float64(0.0)
    for m in res.results:
        total += np.float64(m["pot"].reshape(-1)[0])
    total -= _diag_correction(q)
    return np.array([total], dtype=np.float32), res


def kernel(q, r, cell=None, batch=None):
    out, _ = _run(q, r, trace=False)
    return out
